# revision 24
# baseline (speedup 1.0000x reference)
"""Trainium2 Bass kernel for GQA multi-head attention (TP-8 over heads).

Problem: hidden [1, 4096, 2048] fp32; wq [2048, 2048], wk/wv [2048, 512],
wo [2048, 2048]; 16 q-heads / 4 kv-heads, head_dim 128, interleaved RoPE,
causal softmax attention, o_proj.

Sharding: core c in 0..7 handles q-heads {2c, 2c+1} and kv-head c//2
(kv proj duplicated across core pairs). Each core produces a partial
o_proj output [4096, 2048] (fp16); the host sums the 8 partials in fp32.

v10 structure (measured ~358us vs the 438us v4 baseline; all deltas
below are from real-HW NTFF perfetto traces with per-slice blocked_by):
- hidden is transposed AND quad-tiled on the host: one 1MB DMA loads 4
  k-tiles ([128, 4, 1024] row-major contiguous). DMA *instructions*
  cost ~0.6us each in trigger processing regardless of size, and one
  instruction's 2KB descriptors spread over all 16 HW queues -- so few
  big DMAs beat many small ones (v4's transposing DMAs: 1.2us each,
  serialized; per-k plain loads: ~3us/tile effective).
- bootstrap: whole-weight DMAs + 4 quad DMAs, then chunk 0 projected
  (pass A) and chunk 1 (pass B) on separate PSUM banks; no warmup
  matmuls. Chunk-0 attention starts ~35us earlier than v4.
- denominator finalize (gpsimd partition-reduce; then reciprocal + an2
  normalize) is emitted as units woven into the NEXT chunk (reduce at
  slot 0, recip mid-chunk): v4 ran it at the chunk boundary, jamming
  the DVE FIFO exactly when the next chunk's o_proj psum drains needed
  it, which stalled the in-order PE queue 4-7us per chunk.
- o_proj (po) psum drains all on ACT: the ps_o ring has one bank, so
  the next po matmul blocks on the previous drain; at the po weave
  points the DVE FIFO lags 4-6us while ACT has slack.
- attn-accumulator (au) drains on DVE so ACT reaches the next chunk's
  first exp sooner; the next chunk's first scores matmul is hoisted
  before the current chunk's last PV for the same reason.
- den accumulated in bf16 (same dtype as pt): DVE's 2x 16-bit path
  needs matching dtypes (mixed f16/bf16 adds measured ~1us vs ~0.45us).
- gpsimd elementwise is ~2x slower than DVE and partition_all_reduce
  costs ~6.7us (10x the cost-model estimate) -- keep gpsimd to the
  mask selects + deferred reduce only; never put den adds there.
"""

import sys

sys.path.insert(0, "/opt/trn_rl_repo")

import math

import numpy as np

NUM_HEADS = 16
NUM_KV = 4
HD = 128
H = 2048
KVD = 512
ROPE_BASE = 10000.0
S_FULL = 4096
N_CORES = 8
CH = 512  # chunk of sequence positions handled per attention block
KT = H // 128  # 16 contraction tiles

# rope-swap layout: within each head's 128 dims, pair i=(q*16+r) has its
# even element at partition q*32+r and odd element at q*32+16+r, so the
# swap is a 32-lane stream_shuffle by +-16.
SWAP_MASK = list(range(16, 32)) + list(range(0, 16))


def _rope_perm():
    perm = np.zeros(128, dtype=np.int64)
    for qd in range(4):
        for r in range(16):
            i = qd * 16 + r
            perm[qd * 32 + r] = 2 * i
            perm[qd * 32 + 16 + r] = 2 * i + 1
    return perm


def _rope_tables_T(S):
    import ml_dtypes

    inv = 1.0 / (ROPE_BASE ** (np.arange(0, HD, 2, dtype=np.float64) / HD))
    ang = np.outer(inv, np.arange(S, dtype=np.float64))  # [64, S]
    cosT = np.zeros((128, S))
    sinT = np.zeros((128, S))
    for qd in range(4):
        for r in range(16):
            i = qd * 16 + r
            cosT[qd * 32 + r] = np.cos(ang[i])
            cosT[qd * 32 + 16 + r] = np.cos(ang[i])
            sinT[qd * 32 + r] = -np.sin(ang[i])
            sinT[qd * 32 + 16 + r] = np.sin(ang[i])
    return cosT.astype(ml_dtypes.bfloat16), sinT.astype(ml_dtypes.bfloat16)


def build(S=S_FULL):
    import concourse.bacc as bacc
    import concourse.mybir as mybir
    import concourse.tile as tile
    from concourse import bass_isa

    f32 = mybir.dt.float32
    bf16 = mybir.dt.bfloat16
    f16 = mybir.dt.float16
    AF = mybir.ActivationFunctionType
    ALU = mybir.AluOpType

    NCH = S // CH
    scale = 1.0 / math.sqrt(HD)

    nc = bacc.Bacc("TRN2", target_bir_lowering=False, debug=False, num_devices=N_CORES)

    NPAIR = S // (2 * CH)
    # quad-tiled layout: row-block (p*4+j) of 128 rows holds, for partition
    # part, the 4 k-tiles 4j..4j+3 of hidden[1024p:1024(p+1)].T laid out
    # [part, kq, c] -- one 1MB DMA per quad, 8KB contiguous per partition
    hid = nc.dram_tensor("hiddenT", [NPAIR * 4 * 128, 4 * 2 * CH], bf16,
                         kind="ExternalInput")
    wq = nc.dram_tensor("wq_s", [128, KT * 2 * HD], bf16, kind="ExternalInput")
    wk = nc.dram_tensor("wk_s", [128, KT * HD], bf16, kind="ExternalInput")
    wv = nc.dram_tensor("wv_s", [128, KT * HD], bf16, kind="ExternalInput")
    wo = nc.dram_tensor("wo_s", [128, 2 * H], bf16, kind="ExternalInput")
    out = nc.dram_tensor("out_part", [S, H], f16, kind="ExternalOutput")

    cosT_np, sinT_np = _rope_tables_T(S)
    cos_d = nc.inline_tensor(cosT_np, name="cos_tab")
    sin_d = nc.inline_tensor(sinT_np, name="sin_tab")

    with tile.TileContext(nc) as tc:
        with (
            tc.tile_pool(name="pers", bufs=1) as pers,
            tc.tile_pool(name="hT", bufs=8) as hTp,
            tc.tile_pool(name="pt", bufs=8) as ptp,
            tc.tile_pool(name="den", bufs=2) as denp,
            tc.tile_pool(name="denb", bufs=2) as denbp,
            tc.tile_pool(name="rope", bufs=2) as ropep,
            tc.tile_pool(name="an2", bufs=3) as an2p,
            tc.tile_pool(name="vT", bufs=2) as vTp,
            tc.tile_pool(name="vnat", bufs=8) as vnatp,
            tc.tile_pool(name="au", bufs=2) as aup,
            tc.tile_pool(name="ost", bufs=2) as ostp,
            tc.tile_pool(name="ps_sc", bufs=2, space="PSUM") as ps_sc,
            tc.tile_pool(name="ps_acc", bufs=2, space="PSUM") as ps_acc,
            tc.tile_pool(name="ps_proj", bufs=1, space="PSUM") as ps_proj,
            tc.tile_pool(name="ps_o", bufs=1, space="PSUM") as ps_o,
        ):
            qt0 = pers.tile([128, S], bf16, tag="qt0")
            qt1 = pers.tile([128, S], bf16, tag="qt1")
            kt = pers.tile([128, S], bf16, tag="kt")

            wq_sb = pers.tile([128, KT, 2 * HD], bf16, tag="wq")
            wk_sb = pers.tile([128, KT, HD], bf16, tag="wk")
            wv_sb = pers.tile([128, KT, HD], bf16, tag="wv")
            wo_sb = pers.tile([128, 2, H], bf16, tag="wo")
            cos_sb = pers.tile([128, S], bf16, tag="cos")
            sin_sb = pers.tile([128, S], bf16, tag="sin")
            ones_m = pers.tile([128, 128], bf16, tag="ones")
            nc.vector.memset(ones_m[:], 1.0)

            def load_hT_quad(p, j):
                # one 1MB DMA per 4 k-tiles: DMA *instructions* cost ~0.6us
                # each in trigger processing regardless of size (v8/v9
                # cadence data), while descriptors spread over all 16 HW
                # queues -- so fewer, bigger DMAs win
                t = hTp.tile(
                    [128, 4, 2 * CH], bf16, tag="hTq", name=f"hTq_p{p}_j{j}"
                )
                base = (p * 4 + j) * 128
                nc.sync.dma_start(t[:], hid.ap()[base : base + 128, :])
                return t

            def rope_drain(ps_ap, dst, c):
                # dst[:, CH*c:CH*(c+1)] = psum*cosT + shuffle(psum)*sinT
                # (stream_shuffle requires same src/dst dtype -> f32 xsw)
                xsw = ropep.tile([128, CH], f32, tag="xsw")
                t1 = ropep.tile([128, CH], bf16, tag="t1")
                t2 = ropep.tile([128, CH], bf16, tag="t2")
                nc.vector.stream_shuffle(xsw[:], ps_ap, SWAP_MASK)
                nc.vector.scalar_tensor_tensor(
                    t1[:], ps_ap, 1.0, cos_sb[:, CH * c : CH * (c + 1)],
                    op0=ALU.mult, op1=ALU.mult,
                )
                nc.vector.tensor_mul(t2[:], xsw[:], sin_sb[:, CH * c : CH * (c + 1)])
                nc.vector.tensor_add(dst[:, CH * c : CH * (c + 1)], t1[:], t2[:])

            vnat_tiles = {}

            def v_drain(ps_ap, c):
                vT = vTp.tile([128, CH], bf16, tag="vT", name=f"vT_{c}")
                nc.scalar.copy(vT[:], ps_ap)
                vn = vnatp.tile([128, 4, HD], bf16, tag="vnat", name=f"vnat_{c}")
                vnat_tiles[c] = vn
                for jj in range(4):
                    nc.sync.dma_start_transpose(
                        vn[:, jj, :],
                        vT[:, 128 * jj : 128 * (jj + 1)],
                    )

            # ---- bootstrap: JIT per-k loads; pass A projects chunk 0 on 4
            # accumulators, pass B chunk 1, so chunk-0 drains (and thus
            # chunk-0 attention) start ~14us earlier than a paired k-loop.
            scA = ps_sc.tile([128, 1024], f32, tag="sc", name="boot_qA")
            scB = ps_sc.tile([128, 1024], f32, tag="sc", name="boot_kv")
            bq0 = ps_acc.tile([128, 512], f32, tag="acc", name="boot1_q0")
            bq1 = ps_acc.tile([128, 512], f32, tag="acc", name="boot1_q1")
            bk = ps_proj.tile([128, 512], f32, tag="proj", name="boot1_k")
            bv = ps_o.tile([128, 512], f32, tag="o", name="boot1_v")
            bootA = [
                (scA[:, 0:512], wq_sb, 0, 128),
                (scA[:, 512:1024], wq_sb, 128, 256),
                (scB[:, 0:512], wk_sb, 0, 128),
                (scB[:, 512:1024], wv_sb, 0, 128),
            ]
            bootB = [
                (bq0[:], wq_sb, 0, 128),
                (bq1[:], wq_sb, 128, 256),
                (bk[:], wk_sb, 0, 128),
                (bv[:], wv_sb, 0, 128),
            ]
            nc.sync.dma_start(wq_sb[:], wq.ap())
            nc.sync.dma_start(wk_sb[:], wk.ap())
            nc.sync.dma_start(wv_sb[:], wv.ap())
            hT01 = []
            for j in range(4):
                hT01.append(load_hT_quad(0, j))
            for k in range(KT):
                for ps_ap, wt, lo, hi in bootA:
                    nc.tensor.matmul(
                        ps_ap, wt[:, k, lo:hi], hT01[k // 4][:, k % 4, 0:512],
                        start=(k == 0), stop=(k == KT - 1),
                        skip_group_check=True,
                    )
            # cos/sin head (chunks 0-1) -- needed by the pass-A rope drains
            nc.sync.dma_start(cos_sb[:, : 2 * CH], cos_d.ap()[:, : 2 * CH])
            nc.sync.dma_start(sin_sb[:, : 2 * CH], sin_d.ap()[:, : 2 * CH])
            # pass B (chunk 1) -- overlaps the pass-A drains below
            for k in range(KT):
                for ps_ap, wt, lo, hi in bootB:
                    nc.tensor.matmul(
                        ps_ap, wt[:, k, lo:hi], hT01[k // 4][:, k % 4, 512:1024],
                        start=(k == 0), stop=(k == KT - 1),
                        skip_group_check=True,
                    )
            # pair-1 tiles + remaining tables BEFORE any v_drain: the sync
            # FIFO otherwise gates these pure loads behind the v transposes
            # (which wait on the pass-A/B psum drains)
            hT_next = [load_hT_quad(1, j) for j in range(4)] if NCH > 2 else None
            nc.sync.dma_start(
                cos_sb[:, 2 * CH : 4 * CH], cos_d.ap()[:, 2 * CH : 4 * CH]
            )
            nc.sync.dma_start(
                sin_sb[:, 2 * CH : 4 * CH], sin_d.ap()[:, 2 * CH : 4 * CH]
            )
            rope_drain(scB[:, 0:512], kt, 0)
            rope_drain(scA[:, 0:512], qt0, 0)
            rope_drain(scA[:, 512:1024], qt1, 0)
            v_drain(scB[:, 512:1024], 0)
            rope_drain(bk[:], kt, 1)
            rope_drain(bq0[:], qt0, 1)
            rope_drain(bq1[:], qt1, 1)
            v_drain(bv[:], 1)
            nc.sync.dma_start(wo_sb[:], wo.ap())
            nc.sync.dma_start(cos_sb[:, 4 * CH :], cos_d.ap()[:, 4 * CH :])
            nc.sync.dma_start(sin_sb[:, 4 * CH :], sin_d.ap()[:, 4 * CH :])

            def proj_units(c, hTt, half):
                # 16 subgroup closures (4 dtiles x 4 k-subgroups of 4)
                units = []
                dsts = [
                    (wq_sb, 0, 128, "q0"), (wq_sb, 128, 256, "q1"),
                    (wk_sb, 0, 128, "k"), (wv_sb, 0, 128, "v"),
                ]
                state = {}
                pj_pools = [(ps_proj, "proj"), (ps_o, "o")]
                for d in range(4):
                    wt, lo, hi, nm = dsts[d]
                    for g in range(4):
                        def u(d=d, g=g, wt=wt, lo=lo, hi=hi, nm=nm):
                            if g == 0:
                                pool, tg = pj_pools[d % 2]
                                state[d] = pool.tile(
                                    [128, CH], f32, tag=tg, name=f"pj_{c}_{nm}"
                                )
                            ps = state[d]
                            for k in range(4 * g, 4 * g + 4):
                                nc.tensor.matmul(
                                    ps[:], wt[:, k, lo:hi],
                                    hTt[k // 4][
                                        :, k % 4,
                                        512 * half : 512 * (half + 1),
                                    ],
                                    start=(k == 0), stop=(k == KT - 1),
                                    skip_group_check=True,
                                )
                            if g == 3:
                                if d == 0:
                                    rope_drain(ps[:], qt0, c)
                                elif d == 1:
                                    rope_drain(ps[:], qt1, c)
                                elif d == 2:
                                    rope_drain(ps[:], kt, c)
                                else:
                                    v_drain(ps[:], c)
                        units.append(u)
                return units

            def oproj_units(c, tail=False):
                # o_proj of chunk c from normalized attn an2 [128, 2, 512]
                # (an2_hist[c] is resolved lazily: the fin_norm unit that
                # creates it is woven earlier into the same chunk)
                units = []
                state = {}
                tail_pools = [
                    (ps_o, "o"), (ps_proj, "proj"),
                    (ps_acc, "acc"), (ps_acc, "acc"),
                ]
                for t in range(4):
                    for nn in range(4):
                        def u(t=t, nn=nn, c=c):
                            an2 = an2_hist[c]
                            if nn == 0:
                                state[t] = ostp.tile(
                                    [128, H], f16, tag="ost", name=f"ost_{c}_{t}"
                                )
                            ost = state[t]
                            pool, tg = (
                                tail_pools[(4 * t + nn) % 4] if tail else (ps_o, "o")
                            )
                            po = pool.tile(
                                [128, 512], f32, tag=tg,
                                name=f"po_{c}_{t}_{nn}",
                            )
                            nc.tensor.matmul(
                                po[:], an2[:, 0, 128 * t : 128 * (t + 1)],
                                wo_sb[:, 0, 512 * nn : 512 * (nn + 1)],
                                start=True, stop=False, skip_group_check=True,
                            )
                            nc.tensor.matmul(
                                po[:], an2[:, 1, 128 * t : 128 * (t + 1)],
                                wo_sb[:, 1, 512 * nn : 512 * (nn + 1)],
                                start=False, stop=True, skip_group_check=True,
                            )
                            # all po drains on ACT: at the po weave points the
                            # DVE FIFO lags 4-6us while ACT has slack, and the
                            # in-order PE queue blocks on the ps_o WAR
                            nc.scalar.copy(
                                ost[:, 512 * nn : 512 * (nn + 1)], po[:]
                            )
                            if tail and nn == 1:
                                g = 4 * c + t
                                nc.sync.dma_start(
                                    out.ap()[128 * g : 128 * (g + 1), 0:1024],
                                    ost[:, 0:1024],
                                )
                            elif nn == 3:
                                g = 4 * c + t
                                if tail:
                                    nc.sync.dma_start(
                                        out.ap()[128 * g : 128 * (g + 1), 1024:],
                                        ost[:, 1024:],
                                    )
                                else:
                                    nc.sync.dma_start(
                                        out.ap()[128 * g : 128 * (g + 1), :],
                                        ost[:],
                                    )
                        units.append(u)
                return units

            # deferred denominator finalize: den partition-reduce on gpsimd
            # (emitted early in the NEXT chunk), then recip + an2 normalize
            # (emitted mid/late so the DVE FIFO never waits on the reduce).
            fin_state = {}

            def fin_reduce_unit(i, den):
                def u(i=i, den=den):
                    denb = denbp.tile([128, 1024], f32, tag="denb", name=f"denb_{i}")
                    nc.gpsimd.partition_all_reduce(
                        denb[:], den[:], 128, bass_isa.ReduceOp.add
                    )
                    fin_state[i] = denb
                return u

            def fin_norm_unit(i, au0, au1):
                def u(i=i, au0=au0, au1=au1):
                    denb = fin_state.pop(i)
                    rcb = denbp.tile([128, 1024], f32, tag="rcb", name=f"rcb_{i}")
                    nc.vector.reciprocal_approx_fast(rcb[:], denb[:])
                    an2 = an2p.tile([128, 2, 512], bf16, tag="an2", name=f"an2_{i}")
                    nc.vector.tensor_mul(an2[:, 0, :], au0[:], rcb[:, 0:512])
                    nc.vector.tensor_mul(an2[:, 1, :], au1[:], rcb[:, 512:1024])
                    an2_hist[i] = an2
                return u

            # ---- main loop: attention chunk i with interleaved background
            pair_tiles = {0: hT01, 1: hT_next}
            an2_hist = {}
            sc_pending = {}  # (chunk, kk) -> psum tile (scores issued ahead)

            def issue_scores(ci, kk):
                diag = kk >= 4 * ci
                qoff = 128 * (kk - 4 * ci) if diag else 0
                ktile = kt[:, 128 * kk : 128 * (kk + 1)]
                sc = ps_sc.tile([128, 1024], f32, tag="sc", name=f"sc_{ci}_{kk}")
                nc.tensor.matmul(
                    sc[:, qoff:512], ktile,
                    qt0[:, 512 * ci + qoff : 512 * (ci + 1)],
                    start=True, stop=True, skip_group_check=True,
                )
                nc.tensor.matmul(
                    sc[:, 512 + qoff : 1024], ktile,
                    qt1[:, 512 * ci + qoff : 512 * (ci + 1)],
                    start=True, stop=True, skip_group_check=True,
                )
                sc_pending[(ci, kk)] = sc

            pend_fin = None  # (i-1, den, au0, au1) awaiting finalize
            for i in range(NCH):
                nk = 4 * (i + 1)
                pj = []
                ou = []
                if 1 <= i and i + 1 < NCH:
                    pj = proj_units(i + 1, pair_tiles[(i + 1) // 2], (i + 1) % 2)
                if i in (1, 3):
                    p_new = i // 2 + 2
                    if p_new <= (NCH - 1) // 2:
                        def u_load_a(p=p_new):
                            pair_tiles[p] = [load_hT_quad(p, j) for j in (0, 1)]
                        def u_load_b(p=p_new):
                            pair_tiles[p] += [load_hT_quad(p, j) for j in (2, 3)]
                        pj.append(u_load_a)
                        pj.append(u_load_b)
                if i >= 2:
                    ou = oproj_units(i - 2)
                if i == NCH - 1:
                    # the last chunk is long enough to also absorb o_proj of
                    # chunk NCH-2 (its an2 is ready a few us into this chunk)
                    ou = ou + oproj_units(i - 1)
                # interleave: a proj dtile-group (4 units) then 2 o-units, so
                # consecutive users of each PSUM bank arrive well after the
                # previous user's (queue-lagged) DVE/ACT reads complete
                units = []
                if pend_fin is not None:
                    fi, fden, fau0, fau1 = pend_fin
                    units.append(fin_reduce_unit(fi, fden))
                while pj or ou:
                    units.extend(pj[:4])
                    del pj[:4]
                    units.extend(ou[:2])
                    del ou[:2]
                if pend_fin is not None:
                    # recip+muls: mid-chunk once the chunk is long enough for
                    # the gpsimd reduce (~7us) to have finished; end otherwise
                    slot = len(units) // 2 if i >= 3 else len(units)
                    units.insert(slot, fin_norm_unit(fi, fau0, fau1))
                    pend_fin = None

                acc0 = ps_acc.tile([128, 512], f32, tag="acc", name=f"acc0_{i}")
                acc1 = ps_acc.tile([128, 512], f32, tag="acc", name=f"acc1_{i}")
                # den in bf16 (same dtype as pt): the DVE 2x 16-bit path
                # needs matching dtypes -- mixed f16/bf16 adds ran ~1us,
                # same-dtype ~0.45us (v7 trace)
                den = denp.tile([128, 1024], bf16, tag="den", name=f"den_{i}")

                pend_den = []

                def _flush_den(nflush, den=den, pend_den=pend_den):
                    for _ in range(nflush):
                        kkd, qoffd, ptd = pend_den.pop(0)
                        if kkd == 0:
                            nc.vector.tensor_copy(den[:], ptd[:])
                        elif qoffd == 0:
                            nc.vector.tensor_add(den[:], den[:], ptd[:])
                        else:
                            den3 = den[:].rearrange("p (h q) -> p h q", h=2)[
                                :, :, qoffd:512
                            ]
                            pt3d = ptd[:].rearrange("p (h q) -> p h q", h=2)[
                                :, :, qoffd:512
                            ]
                            nc.vector.tensor_add(den3, den3, pt3d)

                U = len(units)
                emitted = 0
                if (i, 0) not in sc_pending:
                    issue_scores(i, 0)
                # front-load the first proj dtile group: its rope drain then
                # sits near the FRONT of the DVE FIFO (ahead of the den adds)
                # so the next dtile's psum WAR clears ~5us sooner
                while emitted < min(5, U):
                    units[emitted]()
                    emitted += 1
                for kk in range(nk):
                    diag = kk >= 4 * i
                    qoff = 128 * (kk - 4 * i) if diag else 0
                    # scores one iteration ahead: the PE works on sc(kk+1)
                    # while ACT computes exp(kk), so PV(kk) doesn't stall.
                    # On the last iteration, issue the NEXT chunk's first
                    # scores so its exp clears ACT before the boundary.
                    if kk + 1 < nk:
                        issue_scores(i, kk + 1)
                    elif i + 1 < NCH:
                        issue_scores(i + 1, 0)
                    sc = sc_pending.pop((i, kk))
                    pt = ptp.tile([128, 1024], bf16, tag="pt", name=f"pt_{i}_{kk}")
                    if qoff == 0:
                        nc.scalar.activation(pt[:], sc[:], AF.Exp, scale=scale)
                    else:
                        sc3 = sc[:].rearrange("p (h q) -> p h q", h=2)[:, :, qoff:512]
                        pt3 = pt[:].rearrange("p (h q) -> p h q", h=2)[:, :, qoff:512]
                        nc.scalar.activation(pt3, sc3, AF.Exp, scale=scale)
                    if diag:
                        # mask the 128-wide triangle: keep iff (col - qoff) >= p
                        ptm = pt[:].rearrange("p (h q) -> p h q", h=2)[
                            :, :, qoff : qoff + 128
                        ]
                        nc.gpsimd.affine_select(
                            ptm, ptm, [[0, 2], [1, 128]], ALU.is_ge, 0.0,
                            base=0, channel_multiplier=-1,
                        )
                    # weave background units between scores and PV: the PE
                    # chews on them while ACT computes exp(kk). Pace to finish
                    # a step early so unit drains don't jam the boundary.
                    target = (U * (kk + 1) + nk - 2) // max(nk - 1, 1)
                    while emitted < min(target, U):
                        units[emitted]()
                        emitted += 1
                    vtile = vnat_tiles[kk // 4][:, kk % 4, :]
                    nc.tensor.matmul(
                        acc0[:, qoff:512], vtile, pt[:, qoff:512],
                        start=(kk == 0), stop=(kk == nk - 1), skip_group_check=True,
                    )
                    nc.tensor.matmul(
                        acc1[:, qoff:512], vtile, pt[:, 512 + qoff : 1024],
                        start=(kk == 0), stop=(kk == nk - 1), skip_group_check=True,
                    )
                    pend_den.append((kk, qoff, pt))
                    if len(pend_den) > 2:
                        _flush_den(1)
                while emitted < U:
                    units[emitted]()
                    emitted += 1
                _flush_den(len(pend_den))

                # chunk-boundary: drain the attn accumulators (frees the acc
                # psum banks for the next chunk); normalization is deferred.
                # Both drains on DVE: ACT must get to the next chunk's exp.
                au0 = aup.tile([128, 512], bf16, tag="au", name=f"au0_{i}")
                au1 = aup.tile([128, 512], bf16, tag="au", name=f"au1_{i}")
                nc.vector.tensor_copy(au0[:], acc0[:])
                nc.vector.tensor_copy(au1[:], acc1[:])
                if i == NCH - 1:
                    # tail: PE ones-matmul reduce (scores PSUM is free now);
                    # an2 needed immediately by the trailing o_proj units
                    dps = ps_sc.tile([128, 1024], f32, tag="sc", name="dps_tail")
                    nc.tensor.matmul(
                        dps[:, 0:512], ones_m[:], den[:, 0:512],
                        start=True, stop=True, skip_group_check=True,
                    )
                    nc.tensor.matmul(
                        dps[:, 512:1024], ones_m[:], den[:, 512:1024],
                        start=True, stop=True, skip_group_check=True,
                    )
                    rcb = denbp.tile([128, 1024], f32, tag="rcb", name=f"rcb_{i}")
                    nc.vector.reciprocal_approx_fast(rcb[:], dps[:])
                    an2 = an2p.tile([128, 2, 512], bf16, tag="an2", name=f"an2_{i}")
                    nc.vector.tensor_mul(an2[:, 0, :], au0[:], rcb[:, 0:512])
                    nc.vector.tensor_mul(an2[:, 1, :], au1[:], rcb[:, 512:1024])
                    an2_hist[i] = an2
                else:
                    pend_fin = (i, den, au0, au1)

            # trailing o_proj for the last chunk, rotating over free banks
            for u in oproj_units(NCH - 1, tail=True):
                u()

    nc.compile()
    return nc


_CACHE = {}


def _get_program(S=S_FULL):
    if S not in _CACHE:
        _CACHE[S] = build(S)
    return _CACHE[S]


def shard_inputs(hidden_states, wq, wk, wv, wo):
    import ml_dtypes

    bf = ml_dtypes.bfloat16
    hidden_states = np.asarray(hidden_states)
    wq = np.asarray(wq)
    wk = np.asarray(wk)
    wv = np.asarray(wv)
    wo = np.asarray(wo)
    S = hidden_states.shape[1]
    npair = S // (2 * CH)
    # quad-tiled layout (see build()): [p, j, part, kq, c] row-major
    hidT = np.ascontiguousarray(
        hidden_states.reshape(S, H).astype(bf)
        .reshape(npair, 2 * CH, 4, 4, 128)
        .transpose(0, 2, 4, 3, 1)
        .reshape(npair * 4 * 128, 4 * 2 * CH)
    )
    perm = _rope_perm()
    in_maps = []
    for c in range(N_CORES):
        g = c // 2
        wqs = wq[:, 256 * c : 256 * (c + 1)].astype(bf)
        # per-head rope permutation of output columns
        wqp = np.empty_like(wqs)
        wqp[:, 0:128] = wqs[:, perm]
        wqp[:, 128:256] = wqs[:, 128 + perm]
        wks = wk[:, 128 * g : 128 * (g + 1)].astype(bf)[:, perm]
        wvs = wv[:, 128 * g : 128 * (g + 1)].astype(bf)
        wos = wo[256 * c : 256 * (c + 1), :].astype(bf)

        def sbl(w):  # [H, m] -> SBUF layout [128, KT*m]
            m = w.shape[1]
            return np.ascontiguousarray(
                w.reshape(-1, 128, m).transpose(1, 0, 2).reshape(128, -1)
            )

        in_maps.append(
            {
                "hiddenT": hidT,
                "wq_s": sbl(wqp),
                "wk_s": sbl(wks),
                "wv_s": sbl(wvs),
                "wo_s": sbl(wos),
            }
        )
    return in_maps


def kernel(hidden_states, wq, wk, wv, wo, _trace=False):
    from concourse import bass_utils

    B, S, _ = hidden_states.shape
    nc = _get_program(S)
    in_maps = shard_inputs(hidden_states, wq, wk, wv, wo)
    res = bass_utils.run_bass_kernel_spmd(
        nc, in_maps, core_ids=list(range(N_CORES)), trace=_trace
    )
    acc = np.zeros((S, H), dtype=np.float32)
    for c in range(N_CORES):
        acc += res.results[c]["out_part"].astype(np.float32)
    out = acc.reshape(B, S, H)
    if _trace:
        return out, res
    return out
